# revision 1
# baseline (speedup 1.0000x reference)
"""Causal self-attention on 8 Trainium2 NeuronCores.

Problem: B=4, T=2048, C=1024, H=16, DH=64.
  qkv = x @ w_qkv.T ; causal softmax attention per head ; y = attnout @ w_out.T

Sharding: 8 cores = 4 batches x 2 query-subsets. Each core computes the full
QKV projection for its batch (duplicated within the pair -> no collectives),
then attention for a load-balanced set of query rows (all 16 heads), then
the output projection for its own query rows against the full w_out. No
cross-core communication anywhere.

Query balance under causality: global 512-row q-tiles are paired (i, 3-i):
  parity 0 -> q512 tiles [0, 3] (20 key-tiles), parity 1 -> [1, 2] (20).

Everything runs in "transposed space": Q^T/K^T are produced head-pair-stacked
[128=2x64 dh rows, T], scores are computed as S^T (keys on PSUM partitions,
two heads concurrently via PE row-tiling), the softmax denominator is
accumulated on the PE itself (all-ones stationary -> column sums replicated
across all partitions), PV produces attnout^T directly (two heads via PE
column-tiling), and the output projection consumes attnout^T as its
stationary operand — no transposes in any inner loop. All input transposes
(x^T, w^T) are done by DMA-transpose through a bf16 DRAM staging copy.
"""

import threading

import numpy as np

B, T, C = 4, 2048, 1024
H = 16
DH = C // H
P = 128
TL = T // 2          # query rows per core
NPAIR = H // 2       # 8 head-pairs
NCT = C // P         # 8 c-tiles
QT_TILE = 512        # q columns per attention tile
NQT = TL // QT_TILE  # 2 local q-tiles
NEG = -1.0e9

# local q512-tile -> global q512-tile, per parity (also the Q-proj map)
QMAP512 = [[0, 3], [1, 2]]

_cache = {}


def _build_program(parity: int):
    import concourse.mybir as mybir
    import concourse.tile as tile
    from concourse import bacc
    from concourse.masks import make_identity

    f32 = mybir.dt.float32
    bf16 = mybir.dt.bfloat16

    nc = bacc.Bacc("TRN2", target_bir_lowering=False, debug=False)
    x = nc.dram_tensor("x", [T, C], f32, kind="ExternalInput").ap()
    w_qkv = nc.dram_tensor("w_qkv", [3 * C, C], f32, kind="ExternalInput").ap()
    w_out = nc.dram_tensor("w_out", [C, C], f32, kind="ExternalInput").ap()
    y = nc.dram_tensor("y", [TL, C], f32, kind="ExternalOutput").ap()

    g512 = QMAP512[parity]

    with tile.TileContext(nc) as tc:
        with (
            tc.tile_pool(name="res", bufs=1) as res,
            tc.tile_pool(name="stage", bufs=2) as stage,
            tc.tile_pool(name="wtile", bufs=2) as wtile,
            tc.tile_pool(name="work", bufs=2) as work,
            tc.tile_pool(name="rdp", bufs=2) as rdp,
            tc.tile_pool(name="attn", bufs=2) as attnp,
            tc.tile_pool(name="yout", bufs=1) as yout,
        ):
            ones128 = res.tile([P, P], bf16)
            nc.vector.memset(ones128, 1.0)

            # multiplicative bf16 masks for the 4 diagonal k-tiles of a
            # q-tile: keep (1.0) iff col >= row + 128*j, else 0.0
            masks = []
            for j in range(4):
                m = res.tile([P, QT_TILE], bf16, name=f"mask{j}")
                nc.gpsimd.memset(m, 1.0)
                nc.gpsimd.affine_select(
                    out=m, in_=m, compare_op=mybir.AluOpType.is_ge,
                    fill=0.0, base=-P * j, pattern=[[1, QT_TILE]],
                    channel_multiplier=-1,
                )
                masks.append(m)

            ident = res.tile([P, P], bf16)
            make_identity(nc, ident)

            # ---- residents
            kT = res.tile([P, NPAIR, T], bf16)          # K^T   4 MB
            qT = res.tile([P, NPAIR, TL], bf16)         # Q^T   2 MB
            v = res.tile([P, T // P, C], bf16)          # V     4 MB
            wvT = res.tile([P, NCT, C], bf16)           # w_v^T 2 MB
            woT = res.tile([P, NCT, C], bf16)           # w_out^T 2 MB

            with (
                tc.tile_pool(name="xtp", bufs=1) as xtp,
                tc.tile_pool(name="psqkv", bufs=4, space="PSUM") as psqkv,
                tc.tile_pool(name="pst", bufs=2, space="PSUM") as pst,
            ):
                xT = xtp.tile([P, NCT, T], bf16)        # x^T   4 MB

                def load_cast(src_ap):
                    lf = stage.tile([P, C], f32, tag="ldf")
                    nc.sync.dma_start(out=lf, in_=src_ap)
                    lb = stage.tile([P, C], bf16, tag="ldb")
                    nc.any.tensor_copy(out=lb, in_=lf)
                    return lb

                def transpose_block(lb, dst, dst_col):
                    for ct in range(NCT):
                        pt = pst.tile([P, P], bf16, tag="pt")
                        nc.tensor.transpose(
                            pt, lb[:, ct * P:(ct + 1) * P], ident)
                        nc.any.tensor_copy(
                            out=dst[:, ct, dst_col:dst_col + P], in_=pt)

                # x^T via PE transposes (PE busy from ~10us, HAM warms)
                for tt in range(T // P):
                    xb = load_cast(x[tt * P:(tt + 1) * P, :])
                    transpose_block(xb, xT, tt * P)

                # ---- Q/K projections (Q: only local halves)
                for fb in range(16):                     # 0..7 Q, 8..15 K
                    wb = load_cast(w_qkv[fb * P:(fb + 1) * P, :])
                    wqk = wtile.tile([P, NCT, P], bf16, tag="wqk")
                    transpose_block(wb, wqk, 0)
                    if fb < 8:
                        for u in range(NQT):
                            ps = psqkv.tile([P, 512], f32, tag="psqkv")
                            t0 = g512[u] * 512
                            for ct in range(NCT):
                                nc.tensor.matmul(
                                    ps, wqk[:, ct, :],
                                    xT[:, ct, t0:t0 + 512],
                                    start=(ct == 0), stop=(ct == NCT - 1),
                                )
                            nc.vector.tensor_copy(
                                out=qT[:, fb, u * 512:(u + 1) * 512], in_=ps)
                    else:
                        pr = fb - 8
                        for u in range(4):
                            ps = psqkv.tile([P, 512], f32, tag="psqkv")
                            for ct in range(NCT):
                                nc.tensor.matmul(
                                    ps, wqk[:, ct, :],
                                    xT[:, ct, u * 512:(u + 1) * 512],
                                    start=(ct == 0), stop=(ct == NCT - 1),
                                )
                            nc.vector.tensor_copy(
                                out=kT[:, pr, u * 512:(u + 1) * 512], in_=ps)

                # ---- V weights transposed, then V projection
                for fb in range(8):
                    wb = load_cast(w_qkv[(16 + fb) * P:(17 + fb) * P, :])
                    transpose_block(wb, wvT, fb * P)
                for fb in range(8):
                    wb = load_cast(w_out[fb * P:(fb + 1) * P, :])
                    transpose_block(wb, woT, fb * P)
                for tt in range(T // P):
                    for fo in range(2):
                        ps = psqkv.tile([P, 512], f32, tag="psqkv")
                        for ct in range(NCT):
                            nc.tensor.matmul(
                                ps, xT[:, ct, tt * P:(tt + 1) * P],
                                wvT[:, ct, fo * 512:(fo + 1) * 512],
                                start=(ct == 0), stop=(ct == NCT - 1),
                            )
                        nc.vector.tensor_copy(
                            out=v[:, tt, fo * 512:(fo + 1) * 512], in_=ps)

            # ================= attention + out-proj =================
            with (
                tc.tile_pool(name="pss", bufs=2, space="PSUM") as pss,
                tc.tile_pool(name="pso", bufs=4, space="PSUM") as pso,
            ):
                scale = 1.0 / float(np.sqrt(DH))
                for j in range(NQT):
                    G = g512[j]
                    nk = 4 * (G + 1)    # k-tiles (keys 0 .. 512*(G+1))
                    attnT = attnp.tile([P, NPAIR, QT_TILE], bf16, tag="attnT")
                    for p in range(NPAIR):
                        # Two banks per pair: bank A = PV-A (rows 0:64) +
                        # denom-B (rows 64:128); bank B = denom-A (rows 0:64)
                        # + PV-B (rows 64:128). The cross-assignment keeps
                        # every reciprocal/normalize op partition-aligned.
                        poA_t = pso.tile([P, QT_TILE], f32, tag="po",
                                         name="poA")
                        poB_t = pso.tile([P, QT_TILE], f32, tag="po",
                                         name="poB")
                        poA = poA_t[0:64]
                        poB = poB_t[64:128]
                        pdA = poB_t[0:64]
                        pdB = poA_t[64:128]
                        qA = qT[0:64, p, j * QT_TILE:(j + 1) * QT_TILE]
                        qB = qT[64:128, p, j * QT_TILE:(j + 1) * QT_TILE]
                        for k in range(nk):
                            s2 = pss.tile([P, 2, QT_TILE], f32, tag="s2")
                            ks = slice(k * P, (k + 1) * P)
                            nc.tensor.matmul(s2[:, 0, :], kT[0:64, p, ks], qA)
                            nc.tensor.matmul(s2[:, 1, :], kT[64:128, p, ks], qB)
                            p2 = work.tile([P, 2, QT_TILE], bf16, tag="p2")
                            nc.scalar.activation(
                                p2, s2, mybir.ActivationFunctionType.Exp,
                                scale=scale)
                            dj = k - 4 * G
                            if dj >= 0:  # diagonal tile: zero blocked cells
                                nc.vector.tensor_mul(
                                    out=p2, in0=p2,
                                    in1=masks[dj][:, None, :].to_broadcast(
                                        (P, 2, QT_TILE)))
                            pA = p2[:, 0, :]
                            pB = p2[:, 1, :]
                            # the group check is bank-granular but the
                            # pending-zero accounting is partition-exact, so
                            # the partition-split groups are safe to skip-check
                            st, sp_ = (k == 0), (k == nk - 1)
                            nc.tensor.matmul(pdA, ones128[:, 0:64], pA,
                                             start=st, stop=sp_,
                                             skip_group_check=True)
                            nc.tensor.matmul(pdB, ones128[:, 0:64], pB,
                                             start=st, stop=sp_,
                                             skip_group_check=True)
                            nc.tensor.matmul(
                                poA, v[:, k, p * P:p * P + 64], pA,
                                start=st, stop=sp_, skip_group_check=True)
                            nc.tensor.matmul(
                                poB, v[:, k, p * P + 64:(p + 1) * P], pB,
                                start=st, stop=sp_, skip_group_check=True)
                        rD = rdp.tile([P, QT_TILE], f32, tag="rD")
                        nc.vector.reciprocal(rD[0:64], pdA)
                        nc.vector.reciprocal(rD[64:128], pdB)
                        nc.vector.tensor_mul(
                            out=attnT[0:64, p, :], in0=poA, in1=rD[0:64])
                        nc.vector.tensor_mul(
                            out=attnT[64:128, p, :], in0=poB, in1=rD[64:128])
                    # ---- output projection for this q-tile
                    for sub in range(QT_TILE // P):
                        qs = slice(sub * P, (sub + 1) * P)
                        for fo in range(2):
                            ps = pso.tile([P, 512], f32, tag="po", name="psy")
                            for p in range(NPAIR):
                                nc.tensor.matmul(
                                    ps, attnT[:, p, qs],
                                    woT[:, p, fo * 512:(fo + 1) * 512],
                                    start=(p == 0), stop=(p == NPAIR - 1),
                                )
                            ysb = yout.tile([P, 512], f32, tag="ysb")
                            nc.any.tensor_copy(out=ysb, in_=ps)
                            nc.sync.dma_start(
                                out=y[j * QT_TILE + sub * P:
                                      j * QT_TILE + (sub + 1) * P,
                                      fo * 512:(fo + 1) * 512],
                                in_=ysb)

    nc.compile()
    return nc


def _get_program(parity: int):
    if parity not in _cache:
        _cache[parity] = _build_program(parity)
    return _cache[parity]


def _run_group(nc, in_maps, devices, out_holder, idx):
    """shard_map the program over `devices`, one in_map per device."""
    import jax
    from jax.sharding import Mesh, PartitionSpec
    from jax.experimental.shard_map import shard_map
    import concourse.mybir as mybir
    from concourse.bass2jax import (
        _bass_exec_p, install_neuronx_cc_hook, partition_id_tensor)

    install_neuronx_cc_hook()

    partition_name = (
        nc.partition_id_tensor.name if nc.partition_id_tensor else None)
    in_names, out_names, out_avals, zero_outs = [], [], [], []
    for alloc in nc.m.functions[0].allocations:
        if not isinstance(alloc, mybir.MemoryLocationSet):
            continue
        name = alloc.memorylocations[0].name
        if alloc.kind == "ExternalInput":
            if name != partition_name:
                in_names.append(name)
        elif alloc.kind == "ExternalOutput":
            out_names.append(name)
            shape = tuple(alloc.tensor_shape)
            dtype = mybir.dt.np(alloc.dtype)
            out_avals.append(jax.core.ShapedArray(shape, dtype))
            zero_outs.append(np.zeros(shape, dtype))
    n_params = len(in_names)
    n_outs = len(out_avals)
    all_names = in_names + out_names
    if partition_name is not None:
        all_names.append(partition_name)
    donate = tuple(range(n_params, n_params + n_outs))

    def _body(*args):
        operands = list(args)
        if partition_name is not None:
            operands.append(partition_id_tensor())
        outs = _bass_exec_p.bind(
            *operands,
            out_avals=tuple(out_avals),
            in_names=tuple(all_names),
            out_names=tuple(out_names),
            lowering_input_output_aliases=(),
            sim_require_finite=False,
            sim_require_nnan=False,
            nc=nc,
        )
        return tuple(outs)

    n = len(devices)
    mesh = Mesh(np.asarray(devices), ("core",))
    sharded = jax.jit(
        shard_map(
            _body, mesh=mesh,
            in_specs=(PartitionSpec("core"),) * (n_params + n_outs),
            out_specs=(PartitionSpec("core"),) * n_outs,
            check_rep=False,
        ),
        donate_argnums=donate, keep_unused=True,
    )
    concat_in = [
        np.concatenate([np.asarray(m[name]) for m in in_maps], axis=0)
        for name in in_names
    ]
    concat_zero = [
        np.zeros((n * z.shape[0], *z.shape[1:]), z.dtype) for z in zero_outs
    ]
    out_arrs = sharded(*concat_in, *concat_zero)
    out_holder[idx] = [
        {
            name: np.asarray(out_arrs[i]).reshape(n, *out_avals[i].shape)[c]
            for i, name in enumerate(out_names)
        }
        for c in range(n)
    ]


def kernel(x, attn_mask, w_qkv, w_out):
    """Full inputs in, full output out. attn_mask is all-ones (per the
    problem spec) so masking reduces to the causal structure."""
    import jax

    x = np.asarray(x, dtype=np.float32)
    w_qkv = np.asarray(w_qkv, dtype=np.float32)
    w_out = np.asarray(w_out, dtype=np.float32)

    nc_e = _get_program(0)
    nc_o = _get_program(1)

    devices = jax.devices()
    in_maps = [
        {"x": x[b], "w_qkv": w_qkv, "w_out": w_out} for b in range(B)
    ]

    results = [None, None]
    t_e = threading.Thread(
        target=_run_group, args=(nc_e, in_maps, devices[0:4], results, 0))
    t_o = threading.Thread(
        target=_run_group, args=(nc_o, in_maps, devices[4:8], results, 1))
    t_e.start(); t_o.start()
    t_e.join(); t_o.join()

    y = np.empty((B, T, C), dtype=np.float32)
    for parity, group in enumerate(results):
        for b in range(B):
            y_local = group[b]["y"]          # [TL, C] in local q order
            for j in range(NQT):
                G = QMAP512[parity][j]
                y[b, G * QT_TILE:(G + 1) * QT_TILE, :] = \
                    y_local[j * QT_TILE:(j + 1) * QT_TILE, :]
    return y



# revision 22
# speedup vs baseline: 1.6617x; 1.6617x over previous
"""Causal self-attention on 8 Trainium2 NeuronCores.

Problem: B=4, T=2048, C=1024, H=16, DH=64.
  qkv = x @ w_qkv.T ; causal softmax attention per head ; y = attnout @ w_out.T

Sharding: 8 cores = 4 batches x 2 head-halves. Each core computes Q/K/V
projections for its 8 heads over the full sequence, causal attention for
those heads, and a partial output projection (rank-512 contribution).
The host sums the two partial y's per batch (the "all-reduce" of the
tensor-parallel split) — no device collectives.

All inputs are pre-transposed and pre-cast to bf16 on the host, so the
kernel does zero layout transposes and zero dtype-cast passes for its
inputs: x^T, wq^T, wk^T, wv^T, w_out^T stream straight into SBUF.

Attention: scores as S^T (keys on partitions, two heads per pair via PE
quadrant rows), exp on the scalar engine, PV with V as the stationary
operand augmented with a leading ones column — so each PV matmul also
accumulates the softmax denominator into PSUM partition 0 for free.
Normalize = one scalar-engine reciprocal of the denominator row, a
gpsimd partition-broadcast, and one vector multiply per head (fused with
the mandatory PSUM->SBUF copy).

Projection work for key-block u+1 is interleaved instruction-by-
instruction into attention q-tile u to keep the tensor engine dense.
"""

import numpy as np

B, T, C = 4, 2048, 1024
H = 16
DH = 64
P = 128
NCT = C // P         # 8 contract tiles
NPAIR = 4            # per-core head pairs (8 heads)
NJ = 4               # q-tiles of 512
QT = 512

_cache = {}


def _build_program():
    import concourse.mybir as mybir
    import concourse.tile as tile
    from concourse import bacc

    # Pin Exp and Ln to the one activation table set that holds both, so
    # the softmax exps and the ln/exp reciprocal never swap ACT tables.
    # Set ids (positions) are preserved; only membership is filtered.
    _orig_tables = bacc.get_activation_tables
    _EXP = mybir.ActivationFunctionType.Exp
    _LN = mybir.ActivationFunctionType.Ln

    def _pinned_tables(arch):
        t = _orig_tables(arch)
        return {
            name: (fns if name == "natural_log_exp_and_others"
                   else (fns - {_EXP, _LN}))
            for name, fns in t.items()
        }

    bacc.get_activation_tables = _pinned_tables

    f32 = mybir.dt.float32
    bf16 = mybir.dt.bfloat16

    nc = bacc.Bacc("TRN2", target_bir_lowering=False, debug=False)
    xT = nc.dram_tensor("xT", [C, T], bf16, kind="ExternalInput").ap()
    wqT = nc.dram_tensor("wqT", [C, QT], bf16, kind="ExternalInput").ap()
    wkT = nc.dram_tensor("wkT", [C, QT], bf16, kind="ExternalInput").ap()
    wvT = nc.dram_tensor("wvT", [C, QT], bf16, kind="ExternalInput").ap()
    woT = nc.dram_tensor("woT", [QT, C], bf16, kind="ExternalInput").ap()
    y = nc.dram_tensor("y", [T, C], f32, kind="ExternalOutput").ap()

    scale = 1.0 / float(np.sqrt(DH))

    with tile.TileContext(nc) as tc:
        with (
            tc.tile_pool(name="res", bufs=1) as res,
            tc.tile_pool(name="attn", bufs=2) as attnp,
            tc.tile_pool(name="ps", bufs=1, space="PSUM") as psp,
        ):
            # multiplicative bf16 causal mask for the on-diagonal 128x128
            # block: keep (1.0) iff col >= row, else 0.0
            tri = res.tile([P, P], bf16, name="tri")
            nc.gpsimd.memset(tri, 1.0)
            nc.gpsimd.affine_select(
                out=tri, in_=tri, compare_op=mybir.AluOpType.is_ge,
                fill=0.0, base=0, pattern=[[1, P]],
                channel_multiplier=-1,
            )

            # ---- residents
            qTt = res.tile([P, NPAIR, T], bf16)        # Q^T   2 MB
            kTt = res.tile([P, NPAIR, T], bf16)        # K^T   2 MB
            vaug = res.tile([P, T // P, 8, 65], bf16)  # [V | 1s col]
            woTs = res.tile([P, NPAIR, C], bf16)       # w_out^T 1 MB
            xTs = res.tile([P, NCT, T], bf16)          # x^T   4 MB
            wqs = res.tile([P, NCT, QT], bf16)
            wks = res.tile([P, NCT, QT], bf16)
            wvs = res.tile([P, NCT, QT], bf16)

            # trailing ones column for the fused softmax denominator
            nc.vector.memset(vaug, 1.0)
            onesb = res.tile([P, DH], bf16)
            nc.vector.memset(onesb, 1.0)

            # ---- input DMA, in first-consumption order, one transfer per
            # tensor (strided 3D access patterns)
            def w_view(dram, dst, nchunk):
                # dram [nchunk*128, F] -> dst [128, nchunk, F]
                for ct in range(nchunk):
                    nc.sync.dma_start(out=dst[:, ct, :],
                                      in_=dram[ct * P:(ct + 1) * P, :])

            w_view(wkT, wks, NCT)
            for ct in range(NCT):
                nc.sync.dma_start(
                    out=xTs[:, ct, 0:QT],
                    in_=xT[ct * P:(ct + 1) * P, 0:QT])
            w_view(wvT, wvs, NCT)
            w_view(wqT, wqs, NCT)
            for u in range(1, 4):
                for ct in range(NCT):
                    nc.sync.dma_start(
                        out=xTs[:, ct, u * QT:(u + 1) * QT],
                        in_=xT[ct * P:(ct + 1) * P, u * QT:(u + 1) * QT])
            w_view(woT, woTs, NPAIR)

            # ---- projection units as per-instruction thunk lists
            def qk_unit(ws, dst, p, u):
                st = {}

                def mm(ct):
                    def go():
                        if ct == 0:
                            st["ps"] = psp.tile([P, QT], f32, tag="pp",
                                                bufs=2, name="pspj")
                        nc.tensor.matmul(
                            st["ps"], ws[:, ct, p * P:(p + 1) * P],
                            xTs[:, ct, u * QT:(u + 1) * QT],
                            start=(ct == 0), stop=(ct == NCT - 1),
                            skip_group_check=True)
                    return go

                def cp():
                    nc.vector.tensor_copy(
                        out=dst[:, p, u * QT:(u + 1) * QT], in_=st["ps"])
                return [mm(ct) for ct in range(NCT)] + [cp]

            def v_unit(tt):
                st = {}

                def mm(ct):
                    def go():
                        if ct == 0:
                            st["ps"] = psp.tile([P, 8, DH], f32, tag="pp",
                                                bufs=2, name="pspv")
                        nc.tensor.matmul(
                            st["ps"], xTs[:, ct, tt * P:(tt + 1) * P],
                            wvs[:, ct, :],
                            start=(ct == 0), stop=(ct == NCT - 1),
                            skip_group_check=True)
                    return go

                def cp():
                    nc.vector.tensor_copy(
                        out=vaug[:, tt, :, 0:DH], in_=st["ps"])
                return [mm(ct) for ct in range(NCT)] + [cp]

            # ---- attention for one (q-tile, head-pair)
            def attn_pair(j, p, attnT, feed):
                nk = 4 * (j + 1)
                per_kk = -(-len(feed) // nk) if feed else 0
                fi = 0
                pacc = psp.tile([P, 2, QT], f32, tag="pacc", bufs=1,
                                name="pacc")
                for kk in range(nk):
                    # diagonal tiles: only columns q0: are causally valid;
                    # earlier columns keep their prior-kk accumulation
                    dj = kk - 4 * j
                    q0 = dj * P if dj > 0 else 0
                    s2 = psp.tile([P, 2, QT], f32, tag="s2", bufs=2,
                                  name="s2")
                    ks = slice(kk * P, (kk + 1) * P)
                    qs = slice(j * QT + q0, (j + 1) * QT)
                    nc.tensor.matmul(s2[:, 0, q0:], kTt[0:64, p, ks],
                                     qTt[0:64, p, qs],
                                     skip_group_check=True)
                    nc.tensor.matmul(s2[:, 1, q0:], kTt[64:128, p, ks],
                                     qTt[64:128, p, qs],
                                     skip_group_check=True)
                    p2 = attnp.tile([P, 2, QT], bf16, tag="p2", bufs=3,
                                    name="p2")
                    nc.scalar.activation(
                        p2[:, :, q0:], s2[:, :, q0:],
                        mybir.ActivationFunctionType.Exp,
                        scale=scale)
                    if dj >= 0:  # triangular block: zero future cells
                        nc.vector.tensor_mul(
                            out=p2[:, :, q0:q0 + P],
                            in0=p2[:, :, q0:q0 + P],
                            in1=tri[:, None, :].to_broadcast((P, 2, P)))
                    # PV + denominator: out rows 0:64 = P@V, row 64 = sum(p)
                    for h in range(2):
                        nc.tensor.matmul(
                            pacc[0:65, h, q0:], vaug[:, kk, 2 * p + h, :],
                            p2[:, h, q0:],
                            start=(kk == 0), stop=(kk == nk - 1),
                            skip_group_check=True)
                    for fn in feed[fi:fi + per_kk]:
                        fn()
                    fi += per_kk
                for fn in feed[fi:]:
                    fn()
                # normalize; denominator sits on PSUM partition 64.
                # 1/d = exp(-ln d): ACT Reciprocal is blocked for accuracy,
                # and both ln+exp live in one activation table set. All ops
                # keep in/out partition bases equal (engine lanes are
                # partition-locked); only PE and DMA move data across
                # partitions.
                lnrow = attnp.tile([P, 2, QT], f32, tag="lnrow",
                                   name="lnrow")
                nc.scalar.activation(
                    lnrow[64:65, :, :], pacc[64:65, :, :],
                    mybir.ActivationFunctionType.Ln)
                rrow = attnp.tile([P, 2, QT], bf16, tag="rrow", name="rrow")
                nc.scalar.activation(
                    rrow[64:65, :, :], lnrow[64:65, :, :],
                    mybir.ActivationFunctionType.Exp, scale=-1.0)
                # PE broadcast of 1/d to 64 rows at base 0, one bank per head
                bc = psp.tile([P, 2, QT], f32, tag="s2", bufs=2, name="bc")
                for h in range(2):
                    nc.tensor.matmul(
                        bc[0:DH, h, :], onesb[64:65, :],
                        rrow[64:65, h, :], skip_group_check=True)
                bcs = attnp.tile([DH, 2, QT], f32, tag="bcs", name="bcs")
                nc.any.tensor_copy(out=bcs, in_=bc[0:DH, :, :])
                nc.vector.tensor_mul(
                    out=attnT[0:DH, p, :],
                    in0=pacc[0:DH, 0, :], in1=bcs[:, 0, :])
                stgB = attnp.tile([DH, QT], bf16, tag="stgB", name="stgB")
                nc.vector.tensor_mul(
                    out=stgB, in0=pacc[0:DH, 1, :], in1=bcs[:, 1, :])
                # DMA does the partition shift for head B
                nc.sync.dma_start(out=attnT[64:128, p, :], in_=stgB)

            # ---- emit: unit 0 upfront, then attention j with unit j+1
            # interleaved (K/V as feed, Q just before its consumer pair)
            for p in range(NPAIR):
                for fn in qk_unit(wks, kTt, p, 0):
                    fn()
            for tt in range(4):
                for fn in v_unit(tt):
                    fn()
            for p in range(NPAIR):
                for fn in qk_unit(wqs, qTt, p, 0):
                    fn()
            for j in range(NJ):
                attnT = attnp.tile([P, NPAIR, QT], bf16, tag="attnT",
                                   name="attnT")
                for p in range(NPAIR):
                    if j < NJ - 1:
                        feed = (qk_unit(wks, kTt, p, j + 1)
                                + v_unit(4 * (j + 1) + p)
                                + qk_unit(wqs, qTt, p, j + 1))
                    else:
                        feed = []
                    attn_pair(j, p, attnT, feed)
                # out-projection for this q-tile
                for qb in range(4):
                    qs = slice(qb * P, (qb + 1) * P)
                    for fo in range(2):
                        yps = psp.tile([P, QT], f32, tag="pp", bufs=2,
                                       name="yps")
                        for p in range(NPAIR):
                            nc.tensor.matmul(
                                yps, attnT[:, p, qs],
                                woTs[:, p, fo * QT:(fo + 1) * QT],
                                start=(p == 0), stop=(p == NPAIR - 1),
                                skip_group_check=True)
                        ysb = attnp.tile([P, QT], f32, tag="ysb",
                                         name="ysb")
                        nc.vector.tensor_copy(out=ysb, in_=yps)
                        nc.sync.dma_start(
                            out=y[j * QT + qb * P:j * QT + (qb + 1) * P,
                                  fo * QT:(fo + 1) * QT],
                            in_=ysb)

    try:
        nc.compile()
    finally:
        bacc.get_activation_tables = _orig_tables
    return nc


def _get_program():
    if "p" not in _cache:
        _cache["p"] = _build_program()
    return _cache["p"]


def _run_group(nc, in_maps, devices):
    """shard_map the program over `devices`, one in_map per device."""
    import jax
    from jax.sharding import Mesh, PartitionSpec
    from jax.experimental.shard_map import shard_map
    import concourse.mybir as mybir
    from concourse.bass2jax import (
        _bass_exec_p, install_neuronx_cc_hook, partition_id_tensor)

    install_neuronx_cc_hook()

    partition_name = (
        nc.partition_id_tensor.name if nc.partition_id_tensor else None)
    in_names, out_names, out_avals, zero_outs = [], [], [], []
    for alloc in nc.m.functions[0].allocations:
        if not isinstance(alloc, mybir.MemoryLocationSet):
            continue
        name = alloc.memorylocations[0].name
        if alloc.kind == "ExternalInput":
            if name != partition_name:
                in_names.append(name)
        elif alloc.kind == "ExternalOutput":
            out_names.append(name)
            shape = tuple(alloc.tensor_shape)
            dtype = mybir.dt.np(alloc.dtype)
            out_avals.append(jax.core.ShapedArray(shape, dtype))
            zero_outs.append(np.zeros(shape, dtype))
    n_params = len(in_names)
    n_outs = len(out_avals)
    all_names = in_names + out_names
    if partition_name is not None:
        all_names.append(partition_name)
    donate = tuple(range(n_params, n_params + n_outs))

    def _body(*args):
        operands = list(args)
        if partition_name is not None:
            operands.append(partition_id_tensor())
        outs = _bass_exec_p.bind(
            *operands,
            out_avals=tuple(out_avals),
            in_names=tuple(all_names),
            out_names=tuple(out_names),
            lowering_input_output_aliases=(),
            sim_require_finite=False,
            sim_require_nnan=False,
            nc=nc,
        )
        return tuple(outs)

    n = len(devices)
    mesh = Mesh(np.asarray(devices), ("core",))
    sharded = jax.jit(
        shard_map(
            _body, mesh=mesh,
            in_specs=(PartitionSpec("core"),) * (n_params + n_outs),
            out_specs=(PartitionSpec("core"),) * n_outs,
            check_rep=False,
        ),
        donate_argnums=donate, keep_unused=True,
    )
    concat_in = [
        np.concatenate([np.asarray(m[name]) for m in in_maps], axis=0)
        for name in in_names
    ]
    concat_zero = [
        np.zeros((n * z.shape[0], *z.shape[1:]), z.dtype) for z in zero_outs
    ]
    out_arrs = sharded(*concat_in, *concat_zero)
    return [
        {
            name: np.asarray(out_arrs[i]).reshape(n, *out_avals[i].shape)[c]
            for i, name in enumerate(out_names)
        }
        for c in range(n)
    ]


def kernel(x, attn_mask, w_qkv, w_out):
    """Full inputs in, full output out. attn_mask is all-ones (per the
    problem spec) so masking reduces to the causal structure."""
    import jax
    import ml_dtypes

    bf16 = ml_dtypes.bfloat16
    x = np.asarray(x, dtype=np.float32)
    w_qkv = np.asarray(w_qkv, dtype=np.float32)
    w_out = np.asarray(w_out, dtype=np.float32)

    wq, wk, wv = w_qkv[0:C], w_qkv[C:2 * C], w_qkv[2 * C:3 * C]
    xTb = np.ascontiguousarray(x.transpose(0, 2, 1)).astype(bf16)

    in_maps = []
    for core in range(8):
        b, hh = divmod(core, 2)
        sl = slice(hh * QT, (hh + 1) * QT)
        in_maps.append({
            "xT": xTb[b],
            "wqT": np.ascontiguousarray(wq[sl].T).astype(bf16),
            "wkT": np.ascontiguousarray(wk[sl].T).astype(bf16),
            "wvT": np.ascontiguousarray(wv[sl].T).astype(bf16),
            "woT": np.ascontiguousarray(w_out[:, sl].T).astype(bf16),
        })

    nc = _get_program()
    results = _run_group(nc, in_maps, jax.devices()[0:8])

    y = np.empty((B, T, C), dtype=np.float32)
    for b in range(B):
        y[b] = results[2 * b]["y"] + results[2 * b + 1]["y"]
    return y


# revision 31
# speedup vs baseline: 1.6650x; 1.0020x over previous
"""Causal self-attention on 8 Trainium2 NeuronCores.

Problem: B=4, T=2048, C=1024, H=16, DH=64.
  qkv = x @ w_qkv.T ; causal softmax attention per head ; y = attnout @ w_out.T

Sharding: 8 cores = 4 batches x 2 head-halves. Each core computes Q/K/V
projections for its 8 heads over the full sequence, causal attention for
those heads, and a partial output projection (rank-512 contribution).
The host sums the two partial y's per batch (the "all-reduce" of the
tensor-parallel split) — no device collectives.

All inputs are pre-transposed and pre-cast to bf16 on the host, so the
kernel does zero layout transposes and zero dtype-cast passes for its
inputs: x^T, wq^T, wk^T, wv^T, w_out^T stream straight into SBUF.

Attention: scores as S^T (keys on partitions, two heads per pair via PE
quadrant rows), exp on the scalar engine, PV with V as the stationary
operand augmented with a leading ones column — so each PV matmul also
accumulates the softmax denominator into PSUM partition 0 for free.
Normalize = one scalar-engine reciprocal of the denominator row, a
gpsimd partition-broadcast, and one vector multiply per head (fused with
the mandatory PSUM->SBUF copy).

Projection work for key-block u+1 is interleaved instruction-by-
instruction into attention q-tile u to keep the tensor engine dense.
"""

import numpy as np

B, T, C = 4, 2048, 1024
H = 16
DH = 64
P = 128
NCT = C // P         # 8 contract tiles
NPAIR = 4            # per-core head pairs (8 heads)
NJ = 4               # q-tiles of 512
QT = 512

_cache = {}


def _build_program():
    import concourse.mybir as mybir
    import concourse.tile as tile
    from concourse import bacc

    # Pin Exp and Ln to the one activation table set that holds both, so
    # the softmax exps and the ln/exp reciprocal never swap ACT tables.
    # Set ids (positions) are preserved; only membership is filtered.
    _orig_tables = bacc.get_activation_tables
    _EXP = mybir.ActivationFunctionType.Exp
    _LN = mybir.ActivationFunctionType.Ln

    def _pinned_tables(arch):
        t = _orig_tables(arch)
        return {
            name: (fns if name == "natural_log_exp_and_others"
                   else (fns - {_EXP, _LN}))
            for name, fns in t.items()
        }

    bacc.get_activation_tables = _pinned_tables

    f32 = mybir.dt.float32
    bf16 = mybir.dt.bfloat16

    nc = bacc.Bacc("TRN2", target_bir_lowering=False, debug=False)
    xT = nc.dram_tensor("xT", [C, T], bf16, kind="ExternalInput").ap()
    wqT = nc.dram_tensor("wqT", [C, QT], bf16, kind="ExternalInput").ap()
    wkT = nc.dram_tensor("wkT", [C, QT], bf16, kind="ExternalInput").ap()
    wvT = nc.dram_tensor("wvT", [C, QT], bf16, kind="ExternalInput").ap()
    woT = nc.dram_tensor("woT", [QT, C], bf16, kind="ExternalInput").ap()
    y = nc.dram_tensor("y", [T, C], f32, kind="ExternalOutput").ap()

    scale = 1.0 / float(np.sqrt(DH))

    with tile.TileContext(nc) as tc:
        with (
            tc.tile_pool(name="res", bufs=1) as res,
            tc.tile_pool(name="attn", bufs=2) as attnp,
            tc.tile_pool(name="ps", bufs=1, space="PSUM") as psp,
        ):
            # multiplicative bf16 causal mask for the on-diagonal 128x128
            # block: keep (1.0) iff col >= row, else 0.0
            tri = res.tile([P, P], bf16, name="tri")
            nc.gpsimd.memset(tri, 1.0)
            nc.gpsimd.affine_select(
                out=tri, in_=tri, compare_op=mybir.AluOpType.is_ge,
                fill=0.0, base=0, pattern=[[1, P]],
                channel_multiplier=-1,
            )

            # ---- residents
            qTt = res.tile([P, NPAIR, T], bf16)        # Q^T   2 MB
            kTt = res.tile([P, NPAIR, T], bf16)        # K^T   2 MB
            vaug = res.tile([P, T // P, 8, 65], bf16)  # [V | 1s col]
            woTs = res.tile([P, NPAIR, C], bf16)       # w_out^T 1 MB
            xTs = res.tile([P, NCT, T], bf16)          # x^T   4 MB
            wqs = res.tile([P, NCT, QT], bf16)
            wks = res.tile([P, NCT, QT], bf16)
            wvs = res.tile([P, NCT, QT], bf16)

            # trailing ones column for the fused softmax denominator
            nc.vector.memset(vaug, 1.0)
            onesb = res.tile([P, DH], bf16)
            nc.vector.memset(onesb, 1.0)

            # ---- input DMA, in first-consumption order, one transfer per
            # tensor (strided 3D access patterns)
            def w_view(dram, dst, nchunk):
                # dram [nchunk*128, F] -> dst [128, nchunk, F]
                for ct in range(nchunk):
                    nc.sync.dma_start(out=dst[:, ct, :],
                                      in_=dram[ct * P:(ct + 1) * P, :])

            for ct in range(NCT):
                nc.sync.dma_start(out=wks[:, ct, :],
                                  in_=wkT[ct * P:(ct + 1) * P, :])
                nc.sync.dma_start(
                    out=xTs[:, ct, 0:QT],
                    in_=xT[ct * P:(ct + 1) * P, 0:QT])
            w_view(wvT, wvs, NCT)
            w_view(wqT, wqs, NCT)
            for u in range(1, 4):
                for ct in range(NCT):
                    nc.sync.dma_start(
                        out=xTs[:, ct, u * QT:(u + 1) * QT],
                        in_=xT[ct * P:(ct + 1) * P, u * QT:(u + 1) * QT])
            w_view(woT, woTs, NPAIR)

            # ---- projection units as per-instruction thunk lists
            def qk_unit(ws, dst, p, u):
                st = {}

                def mm(ct):
                    def go():
                        if ct == 0:
                            st["ps"] = psp.tile([P, QT], f32, tag="pp",
                                                bufs=2, name="pspj")
                        nc.tensor.matmul(
                            st["ps"], ws[:, ct, p * P:(p + 1) * P],
                            xTs[:, ct, u * QT:(u + 1) * QT],
                            start=(ct == 0), stop=(ct == NCT - 1),
                            skip_group_check=True)
                    return go

                def cp():
                    nc.vector.tensor_copy(
                        out=dst[:, p, u * QT:(u + 1) * QT], in_=st["ps"])
                return [mm(ct) for ct in range(NCT)] + [cp]

            def v_unit(tt):
                st = {}

                def mm(ct):
                    def go():
                        if ct == 0:
                            st["ps"] = psp.tile([P, 8, DH], f32, tag="pp",
                                                bufs=2, name="pspv")
                        nc.tensor.matmul(
                            st["ps"], xTs[:, ct, tt * P:(tt + 1) * P],
                            wvs[:, ct, :],
                            start=(ct == 0), stop=(ct == NCT - 1),
                            skip_group_check=True)
                    return go

                def cp():
                    nc.vector.tensor_copy(
                        out=vaug[:, tt, :, 0:DH], in_=st["ps"])
                return [mm(ct) for ct in range(NCT)] + [cp]

            # ---- attention for one (q-tile, head-pair)
            def attn_pair(j, p, attnT, feed):
                nk = 4 * (j + 1)
                per_kk = -(-len(feed) // nk) if feed else 0
                fi = 0
                pacc = psp.tile([P, 2, QT], f32, tag="pacc", bufs=1,
                                name="pacc")
                for kk in range(nk):
                    # diagonal tiles: only columns q0: are causally valid;
                    # earlier columns keep their prior-kk accumulation
                    dj = kk - 4 * j
                    q0 = dj * P if dj > 0 else 0
                    s2 = psp.tile([P, 2, QT], f32, tag="s2", bufs=2,
                                  name="s2")
                    ks = slice(kk * P, (kk + 1) * P)
                    qs = slice(j * QT + q0, (j + 1) * QT)
                    nc.tensor.matmul(s2[:, 0, q0:], kTt[0:64, p, ks],
                                     qTt[0:64, p, qs],
                                     skip_group_check=True)
                    nc.tensor.matmul(s2[:, 1, q0:], kTt[64:128, p, ks],
                                     qTt[64:128, p, qs],
                                     skip_group_check=True)
                    p2 = attnp.tile([P, 2, QT], bf16, tag="p2", bufs=3,
                                    name="p2")
                    nc.scalar.activation(
                        p2[:, :, q0:], s2[:, :, q0:],
                        mybir.ActivationFunctionType.Exp,
                        scale=scale)
                    if dj >= 0:  # triangular block: zero future cells
                        nc.vector.tensor_mul(
                            out=p2[:, :, q0:q0 + P],
                            in0=p2[:, :, q0:q0 + P],
                            in1=tri[:, None, :].to_broadcast((P, 2, P)))
                    # PV + denominator: out rows 0:64 = P@V, row 64 = sum(p)
                    for h in range(2):
                        nc.tensor.matmul(
                            pacc[0:65, h, q0:], vaug[:, kk, 2 * p + h, :],
                            p2[:, h, q0:],
                            start=(kk == 0), stop=(kk == nk - 1),
                            skip_group_check=True)
                    for fn in feed[fi:fi + per_kk]:
                        fn()
                    fi += per_kk
                for fn in feed[fi:]:
                    fn()
                # normalize; denominator sits on PSUM partition 64.
                # 1/d = exp(-ln d): ACT Reciprocal is blocked for accuracy,
                # and both ln+exp live in one activation table set. All ops
                # keep in/out partition bases equal (engine lanes are
                # partition-locked); only PE and DMA move data across
                # partitions.
                lnrow = attnp.tile([P, 2, QT], f32, tag="lnrow",
                                   name="lnrow")
                nc.scalar.activation(
                    lnrow[64:65, :, :], pacc[64:65, :, :],
                    mybir.ActivationFunctionType.Ln)
                rrow = attnp.tile([P, 2, QT], bf16, tag="rrow", name="rrow")
                nc.scalar.activation(
                    rrow[64:65, :, :], lnrow[64:65, :, :],
                    mybir.ActivationFunctionType.Exp, scale=-1.0)
                # PE broadcast of 1/d to 64 rows at base 0, one bank per head
                bc = psp.tile([P, 2, QT], f32, tag="s2", bufs=2, name="bc")
                for h in range(2):
                    nc.tensor.matmul(
                        bc[0:DH, h, :], onesb[64:65, :],
                        rrow[64:65, h, :], skip_group_check=True)
                bcs = attnp.tile([DH, 2, QT], f32, tag="bcs", name="bcs")
                nc.any.tensor_copy(out=bcs, in_=bc[0:DH, :, :])
                nc.vector.tensor_mul(
                    out=attnT[0:DH, p, :],
                    in0=pacc[0:DH, 0, :], in1=bcs[:, 0, :])
                stgB = attnp.tile([DH, QT], bf16, tag="stgB", name="stgB")
                nc.vector.tensor_mul(
                    out=stgB, in0=pacc[0:DH, 1, :], in1=bcs[:, 1, :])
                # DMA does the partition shift for head B
                nc.sync.dma_start(out=attnT[64:128, p, :], in_=stgB)

            # ---- emit: unit 0 upfront, then attention j with unit j+1
            # interleaved (K/V as feed, Q just before its consumer pair)
            for p in range(NPAIR):
                for fn in qk_unit(wks, kTt, p, 0):
                    fn()
            for tt in range(4):
                for fn in v_unit(tt):
                    fn()
            for p in range(NPAIR):
                for fn in qk_unit(wqs, qTt, p, 0):
                    fn()
            for j in range(NJ):
                attnT = attnp.tile([P, NPAIR, QT], bf16, tag="attnT",
                                   name="attnT")
                for p in range(NPAIR):
                    if j < NJ - 1:
                        feed = (qk_unit(wks, kTt, p, j + 1)
                                + v_unit(4 * (j + 1) + p)
                                + qk_unit(wqs, qTt, p, j + 1))
                    else:
                        feed = []
                    attn_pair(j, p, attnT, feed)
                # out-projection for this q-tile
                for qb in range(4):
                    qs = slice(qb * P, (qb + 1) * P)
                    for fo in range(2):
                        yps = psp.tile([P, QT], f32, tag="pp", bufs=2,
                                       name="yps")
                        for p in range(NPAIR):
                            nc.tensor.matmul(
                                yps, attnT[:, p, qs],
                                woTs[:, p, fo * QT:(fo + 1) * QT],
                                start=(p == 0), stop=(p == NPAIR - 1),
                                skip_group_check=True)
                        ysb = attnp.tile([P, QT], f32, tag="ysb",
                                         name="ysb")
                        nc.vector.tensor_copy(out=ysb, in_=yps)
                        nc.sync.dma_start(
                            out=y[j * QT + qb * P:j * QT + (qb + 1) * P,
                                  fo * QT:(fo + 1) * QT],
                            in_=ysb)

    try:
        nc.compile()
    finally:
        bacc.get_activation_tables = _orig_tables
    return nc


def _get_program():
    if "p" not in _cache:
        _cache["p"] = _build_program()
    return _cache["p"]


def _run_group(nc, in_maps, devices):
    """shard_map the program over `devices`, one in_map per device."""
    import jax
    from jax.sharding import Mesh, PartitionSpec
    from jax.experimental.shard_map import shard_map
    import concourse.mybir as mybir
    from concourse.bass2jax import (
        _bass_exec_p, install_neuronx_cc_hook, partition_id_tensor)

    install_neuronx_cc_hook()

    partition_name = (
        nc.partition_id_tensor.name if nc.partition_id_tensor else None)
    in_names, out_names, out_avals, zero_outs = [], [], [], []
    for alloc in nc.m.functions[0].allocations:
        if not isinstance(alloc, mybir.MemoryLocationSet):
            continue
        name = alloc.memorylocations[0].name
        if alloc.kind == "ExternalInput":
            if name != partition_name:
                in_names.append(name)
        elif alloc.kind == "ExternalOutput":
            out_names.append(name)
            shape = tuple(alloc.tensor_shape)
            dtype = mybir.dt.np(alloc.dtype)
            out_avals.append(jax.core.ShapedArray(shape, dtype))
            zero_outs.append(np.zeros(shape, dtype))
    n_params = len(in_names)
    n_outs = len(out_avals)
    all_names = in_names + out_names
    if partition_name is not None:
        all_names.append(partition_name)
    donate = tuple(range(n_params, n_params + n_outs))

    def _body(*args):
        operands = list(args)
        if partition_name is not None:
            operands.append(partition_id_tensor())
        outs = _bass_exec_p.bind(
            *operands,
            out_avals=tuple(out_avals),
            in_names=tuple(all_names),
            out_names=tuple(out_names),
            lowering_input_output_aliases=(),
            sim_require_finite=False,
            sim_require_nnan=False,
            nc=nc,
        )
        return tuple(outs)

    n = len(devices)
    mesh = Mesh(np.asarray(devices), ("core",))
    sharded = jax.jit(
        shard_map(
            _body, mesh=mesh,
            in_specs=(PartitionSpec("core"),) * (n_params + n_outs),
            out_specs=(PartitionSpec("core"),) * n_outs,
            check_rep=False,
        ),
        donate_argnums=donate, keep_unused=True,
    )
    concat_in = [
        np.concatenate([np.asarray(m[name]) for m in in_maps], axis=0)
        for name in in_names
    ]
    concat_zero = [
        np.zeros((n * z.shape[0], *z.shape[1:]), z.dtype) for z in zero_outs
    ]
    out_arrs = sharded(*concat_in, *concat_zero)
    return [
        {
            name: np.asarray(out_arrs[i]).reshape(n, *out_avals[i].shape)[c]
            for i, name in enumerate(out_names)
        }
        for c in range(n)
    ]


def kernel(x, attn_mask, w_qkv, w_out):
    """Full inputs in, full output out. attn_mask is all-ones (per the
    problem spec) so masking reduces to the causal structure."""
    import jax
    import ml_dtypes

    bf16 = ml_dtypes.bfloat16
    x = np.asarray(x, dtype=np.float32)
    w_qkv = np.asarray(w_qkv, dtype=np.float32)
    w_out = np.asarray(w_out, dtype=np.float32)

    wq, wk, wv = w_qkv[0:C], w_qkv[C:2 * C], w_qkv[2 * C:3 * C]
    xTb = np.ascontiguousarray(x.transpose(0, 2, 1)).astype(bf16)

    in_maps = []
    for core in range(8):
        b, hh = divmod(core, 2)
        sl = slice(hh * QT, (hh + 1) * QT)
        in_maps.append({
            "xT": xTb[b],
            "wqT": np.ascontiguousarray(wq[sl].T).astype(bf16),
            "wkT": np.ascontiguousarray(wk[sl].T).astype(bf16),
            "wvT": np.ascontiguousarray(wv[sl].T).astype(bf16),
            "woT": np.ascontiguousarray(w_out[:, sl].T).astype(bf16),
        })

    nc = _get_program()
    results = _run_group(nc, in_maps, jax.devices()[0:8])

    y = np.empty((B, T, C), dtype=np.float32)
    for b in range(B):
        y[b] = results[2 * b]["y"] + results[2 * b + 1]["y"]
    return y
